# revision 10
# baseline (speedup 1.0000x reference)
"""3-layer GAT on Trainium2 — 8-core SPMD Bass kernel (bf16 pipeline).

Node-partitioned (edge-cut) distribution:
- core c owns nodes [c*NPC, (c+1)*NPC); nodes are re-ordered within each core
  (degree-sorted snake) so the 49 windows of 128 dst nodes carry near-equal
  edge counts.
- per layer: each core computes its nodes' [h | als] rows (bf16) plus a local
  ald tile, publishes rows to a replicated table via AllGather, then processes
  its incoming edges in windows: dma_gather of source rows (bf16),
  exp(leaky(als+ald)) on-chip, scatter-add via one-hot matmul into PSUM,
  normalize by the per-node sum at the end.
- all tables, gathers and matmuls run in bf16 (f32 PSUM accumulation).
"""
from dataclasses import dataclass, field

import numpy as np
import ml_dtypes

import concourse.bass as bass
import concourse.bacc as bacc
import concourse.mybir as mybir
import concourse.tile as tile
from concourse.masks import make_identity

P = 128
N_CORES = 8
NEG_SLOPE = 0.2
DUMMY_ALS = -1000.0

BF16 = ml_dtypes.bfloat16


@dataclass
class LayerCfg:
    fin: int
    fout: int
    H: int
    C: int
    drow: int      # table row width in bf16 elements (multiple of 128)
    relu: bool

    @property
    def ex(self):
        return 2 * self.H

    @property
    def fw(self):
        return self.fout + self.ex


@dataclass
class GatCfg:
    n: int
    n_cores: int = N_CORES
    layers: tuple = ()
    # filled by preprocessing
    ta: tuple = ()          # per-window A-half slot columns (max over cores)
    tb: tuple = ()          # per-window B-half slot columns
    newpos: np.ndarray = None   # global node -> position within its core
    ready: bool = False

    @property
    def npc(self):
        return self.n // self.n_cores

    @property
    def n_win(self):
        return (self.npc + P - 1) // P

    @property
    def pos(self):
        return self.n_win * P

    @property
    def rows(self):
        return self.pos + 1

    @property
    def half(self):
        return self.rows * (self.n_cores // 2)

    @property
    def tbl(self):
        return self.rows * self.n_cores

    @property
    def t_max(self):
        return max(a + b for a, b in zip(self.ta, self.tb))


def real_cfg():
    return GatCfg(
        n=50000,
        layers=(
            LayerCfg(256, 256, 8, 32, 384, True),
            LayerCfg(256, 256, 8, 32, 384, True),
            LayerCfg(256, 64, 1, 64, 128, False),
        ),
    )


def mini_cfg():
    return GatCfg(
        n=1024,
        layers=(
            LayerCfg(256, 256, 8, 32, 384, True),
            LayerCfg(256, 256, 8, 32, 384, True),
            LayerCfg(256, 64, 1, 64, 128, False),
        ),
    )


# ---------------------------------------------------------------- host prep

def _wrap16(idx_list):
    """dma_gather index layout: idx i lives at [i % 16, i // 16]; replicate
    the 16-partition block 8x down to 128 partitions."""
    a = idx_list.reshape(-1, 16).T  # [16, len/16]
    return np.tile(a, (8, 1)).astype(np.int16)


def preprocess(cfg: GatCfg, edge_index):
    """Degree-balanced node->window assignment (snake over degree-sorted
    nodes), then per-window column-major edge packing split by src half.

    Sets cfg.ta/tb/newpos; returns per-core index inputs.
    """
    npc, n_win, n_cores = cfg.npc, cfg.n_win, cfg.n_cores
    src = np.concatenate([np.asarray(edge_index[0], np.int64),
                          np.arange(cfg.n, dtype=np.int64)])
    dst = np.concatenate([np.asarray(edge_index[1], np.int64),
                          np.arange(cfg.n, dtype=np.int64)])
    dst_core = dst // npc

    # --- node placement: snake over degree-sorted nodes, per core
    newpos = np.empty(cfg.n, dtype=np.int64)
    for c in range(n_cores):
        sel = dst_core == c
        d_loc = dst[sel] - c * npc
        deg = np.bincount(d_loc, minlength=npc)
        order = np.argsort(-deg, kind="stable")  # old local, degree desc
        r = np.arange(npc)
        k, j = r // n_win, r % n_win
        w = np.where(k % 2 == 0, j, n_win - 1 - j)
        newpos[c * npc + order] = w * P + k
    cfg.newpos = newpos

    rows, half = cfg.rows, cfg.half
    sidx = (src // npc) * rows + newpos[src]
    is_b = sidx >= half

    # --- per-core edge grouping
    per_core = []
    counts_a = np.zeros((n_cores, n_win), dtype=np.int64)
    counts_b = np.zeros((n_cores, n_win), dtype=np.int64)
    for c in range(n_cores):
        sel = np.nonzero(dst_core == c)[0]
        d_new = newpos[dst[sel]]
        w = d_new // P
        half_flag = is_b[sel].astype(np.int64)
        order = np.lexsort((d_new, half_flag, w))
        sel = sel[order]
        d_new = d_new[order]
        w = w[order]
        half_flag = half_flag[order]
        rel = (sidx[sel] - half_flag * half).astype(np.int64)
        for wi in range(n_win):
            m = w == wi
            counts_a[c, wi] = int((m & (half_flag == 0)).sum())
            counts_b[c, wi] = int((m & (half_flag == 1)).sum())
        per_core.append((d_new % P, w, half_flag, rel))

    ta = np.maximum(1, np.ceil(counts_a.max(axis=0) / P).astype(int))
    tb = np.maximum(1, np.ceil(counts_b.max(axis=0) / P).astype(int))
    cfg.ta, cfg.tb = tuple(int(x) for x in ta), tuple(int(x) for x in tb)
    cfg.ready = True
    t_max = cfg.t_max
    dummy = cfg.pos  # dummy row index, valid in A-rel and B-rel coords

    idx_inputs = []
    for c in range(n_cores):
        d_loc, w, half_flag, rel = per_core[c]
        idx16 = np.zeros((n_win, P, t_max * 8), dtype=np.int16)
        dst32 = np.zeros((n_win, P, t_max), dtype=np.int32)
        for wi in range(n_win):
            tA, tB = cfg.ta[wi], cfg.tb[wi]
            t = tA + tB
            m = w == wi
            ra = rel[m & (half_flag == 0)]
            rb = rel[m & (half_flag == 1)]
            da = d_loc[m & (half_flag == 0)]
            db = d_loc[m & (half_flag == 1)]
            src_list = np.full(t * P, dummy, dtype=np.int64)
            dl_list = np.zeros(t * P, dtype=np.int64)
            src_list[: len(ra)] = ra
            src_list[tA * P : tA * P + len(rb)] = rb
            dl_list[: len(da)] = da
            dl_list[tA * P : tA * P + len(db)] = db
            wa = _wrap16(src_list[: tA * P])
            wb = _wrap16(src_list[tA * P :])
            idx16[wi, :, : t * 8] = np.concatenate([wa, wb], axis=1)
            # dstloc in (p, j) layout: edge i -> [i % 128, i // 128]
            dst32[wi, :, :t] = dl_list.reshape(t, P).T.astype(np.int32)
        idx_inputs.append({"idx16": idx16, "dst32": dst32})
    return idx_inputs


def shard_inputs(cfg: GatCfg, inputs):
    """Build the per-core input dicts for run_bass_kernel_spmd."""
    x = np.asarray(inputs["x"], dtype=np.float32)
    edge_index = np.asarray(inputs["edge_index"])
    idx_inputs = preprocess(cfg, edge_index)

    def blockdiag(a_s, a_d, fin_rows):
        H, C = a_s.shape
        A = np.zeros((fin_rows, 2 * H), dtype=np.float64)
        for h in range(H):
            A[h * C : (h + 1) * C, h] = a_s[h]
            A[h * C : (h + 1) * C, H + h] = a_d[h]
        return A

    # Device activations are stored head-minor ("c-major": feature (h,c) lives
    # at column c*H + h) so the per-edge exp broadcast multiplies with a
    # packed last dim (DVE 2x mode). Weight columns/rows are permuted here.
    weight_common = {}
    pm_prev = None
    for li, nm in enumerate(["1", "2", "3"]):
        lc = cfg.layers[li]
        pm = np.array([h * lc.C + c for c in range(lc.C) for h in range(lc.H)])
        Wn = np.asarray(inputs[f"W{nm}"], dtype=np.float64)
        if pm_prev is not None:
            Wn = Wn[pm_prev, :]
        A = blockdiag(
            np.asarray(inputs[f"as{nm}"], np.float64),
            np.asarray(inputs[f"ad{nm}"], np.float64),
            lc.fout,
        )
        WA = Wn @ A  # [fin, 2H]
        Wx = np.concatenate([Wn[:, pm], WA], axis=1)  # [fin, fout + 2H]
        weight_common[f"Wx{nm}"] = np.ascontiguousarray(Wx.astype(BF16))
        weight_common[f"b{nm}"] = np.asarray(
            inputs[f"b{nm}"], np.float32).reshape(1, -1)[:, pm]
        pm_prev = pm

    in_maps = []
    for c in range(cfg.n_cores):
        xs = np.zeros((cfg.pos, cfg.layers[0].fin), dtype=BF16)
        loc = cfg.newpos[c * cfg.npc : (c + 1) * cfg.npc]
        xs[loc] = x[c * cfg.npc : (c + 1) * cfg.npc].astype(BF16)
        m = {"x_sh": xs, **idx_inputs[c], **weight_common}
        in_maps.append(m)
    return in_maps


def unshard_output(cfg: GatCfg, got):
    """Map the concatenated per-core outputs back to global node order."""
    out = np.empty((cfg.n, got.shape[1]), dtype=got.dtype)
    for c in range(cfg.n_cores):
        loc = cfg.newpos[c * cfg.npc : (c + 1) * cfg.npc]
        out[c * cfg.npc : (c + 1) * cfg.npc] = got[c * cfg.pos + loc]
    return out


# ---------------------------------------------------------------- device code

def build_program(cfg: GatCfg, repeats: int = 1, skip_ag: bool = False):
    assert cfg.ready, "run preprocess/shard_inputs first (sets ta/tb)"
    nc = bacc.Bacc("TRN2", target_bir_lowering=False, debug=False,
                   num_devices=cfg.n_cores)
    n_win, t_max = cfg.n_win, cfg.t_max
    f32 = mybir.dt.float32
    bf16 = mybir.dt.bfloat16

    # ---- I/O
    x_sh = nc.dram_tensor("x_sh", [cfg.pos, cfg.layers[0].fin], bf16,
                          kind="ExternalInput")
    idx16 = nc.dram_tensor("idx16", [n_win, P, t_max * 8], mybir.dt.int16,
                           kind="ExternalInput")
    dst32 = nc.dram_tensor("dst32", [n_win, P, t_max], mybir.dt.int32,
                           kind="ExternalInput")
    wt_in = {}
    for li, nm in enumerate(["1", "2", "3"]):
        lc = cfg.layers[li]
        wt_in[f"Wx{nm}"] = nc.dram_tensor(f"Wx{nm}", [lc.fin, lc.fw], bf16,
                                          kind="ExternalInput")
        wt_in[f"b{nm}"] = nc.dram_tensor(f"b{nm}", [1, lc.fout], f32,
                                         kind="ExternalInput")
    out_t = nc.dram_tensor("out", [cfg.pos, cfg.layers[-1].fout], f32,
                           kind="ExternalOutput")

    # ---- internal DRAM
    hal12_loc = nc.dram_tensor("hal12_loc", [cfg.rows, 384], bf16,
                               kind="Internal")
    hal12_full = nc.dram_tensor("hal12_full", [cfg.tbl, 384], bf16,
                                kind="Internal", addr_space="Shared")
    hal3_loc = nc.dram_tensor("hal3_loc", [cfg.rows, 128], bf16,
                              kind="Internal")
    hal3_full = nc.dram_tensor("hal3_full", [cfg.tbl, 128], bf16,
                               kind="Internal", addr_space="Shared")

    iota_const = nc.inline_tensor(
        np.tile(np.arange(P, dtype=np.float32), (P, 1)), name="iota_const")

    rg = [list(range(cfg.n_cores))]

    with tile.TileContext(nc) as tc:
        import contextlib
        with contextlib.ExitStack() as ctx:
            persist = ctx.enter_context(tc.tile_pool(name="persist", bufs=1))
            wpool = ctx.enter_context(tc.tile_pool(name="wts", bufs=1))
            sb = ctx.enter_context(tc.tile_pool(name="work", bufs=2))
            gp = ctx.enter_context(tc.tile_pool(name="gathp", bufs=3))
            sm = ctx.enter_context(tc.tile_pool(name="small", bufs=4))
            fr = ctx.enter_context(tc.tile_pool(name="front", bufs=2))
            ps = ctx.enter_context(tc.tile_pool(name="psum", bufs=2, space="PSUM"))
            psf = ctx.enter_context(tc.tile_pool(name="psumf", bufs=2, space="PSUM"))

            identity = persist.tile([P, P], bf16)
            make_identity(nc, identity[:])
            iota_f32 = persist.tile([P, P], f32)
            nc.sync.dma_start(out=iota_f32[:], in_=iota_const[:])
            iota_bf = persist.tile([P, P], bf16)
            nc.vector.tensor_copy(out=iota_bf[:], in_=iota_f32[:])

            in_local = persist.tile([P, n_win, cfg.layers[0].fin], bf16)
            for _rep in range(repeats):
              for w in range(n_win):
                nc.sync.dma_start(out=in_local[:, w, :],
                                  in_=x_sh[w * P : (w + 1) * P, :])

              for li in range(len(cfg.layers)):
                  lc = cfg.layers[li]
                  nm = str(li + 1)
                  hal_loc = hal12_loc if lc.drow == 384 else hal3_loc
                  hal_full = hal12_full if lc.drow == 384 else hal3_full
                  kch = lc.fin // P  # input chunks (contraction)

                  # ---------- per-layer constants
                  w_ext = wpool.tile([P, kch, 272], bf16, tag="w_ext")
                  nc.sync.dma_start(
                      out=w_ext[:, :, : lc.fw],
                      in_=wt_in[f"Wx{nm}"][:].rearrange("(q p) f -> p q f", p=P))
                  ald_all = wpool.tile([P, n_win * 8], bf16, tag="ald_all")
                  b_t = sm.tile([1, lc.fout], f32, tag="b_t")
                  nc.sync.dma_start(out=b_t[:], in_=wt_in[f"b{nm}"][:])
                  b_bc = wpool.tile([P, lc.fout], f32, tag="b_bc")
                  nc.gpsimd.partition_broadcast(b_bc[:], b_t[:1, :])

                  # ---------- front phase: [h | als] for own nodes -> hal_loc
                  for w in range(n_win):
                      in_t = fr.tile([P, lc.fin], bf16, tag="in_t")
                      for q in range(kch):
                          tp = psf.tile([P, P], bf16, tag="tp")
                          nc.tensor.transpose(
                              tp[:], in_local[:, w, q * P : (q + 1) * P],
                              identity[:])
                          nc.vector.tensor_copy(
                              out=in_t[:, q * P : (q + 1) * P], in_=tp[:])
                      h_ps = psf.tile([P, 272], f32, tag="h_ps")
                      for q in range(kch):
                          nc.tensor.matmul(
                              h_ps[:, : lc.fw],
                              lhsT=in_t[:, q * P : (q + 1) * P],
                              rhs=w_ext[:, q, : lc.fw],
                              start=(q == 0), stop=(q == kch - 1))
                      nc.scalar.copy(
                          out=ald_all[:, w * lc.H : (w + 1) * lc.H],
                          in_=h_ps[:, lc.fout + lc.H : lc.fout + 2 * lc.H])
                      stage = fr.tile([P, lc.drow], bf16, tag=f"stage{lc.drow}")
                      nc.vector.tensor_copy(
                          out=stage[:, : lc.fout + lc.H],
                          in_=h_ps[:, : lc.fout + lc.H])
                      nc.sync.dma_start(
                          out=hal_loc[w * P : (w + 1) * P, : lc.fout + lc.H],
                          in_=stage[:, : lc.fout + lc.H])
                  # dummy row
                  dmy = sm.tile([1, lc.drow], bf16, tag=f"dmy{lc.drow}")
                  nc.vector.memset(dmy[:], 0.0)
                  nc.vector.memset(
                      dmy[:1, lc.fout : lc.fout + lc.H], DUMMY_ALS)
                  nc.sync.dma_start(out=hal_loc[cfg.pos : cfg.pos + 1, :],
                                    in_=dmy[:])

                  if not skip_ag:
                      nc.gpsimd.collective_compute(
                          "AllGather", mybir.AluOpType.bypass,
                          ins=[hal_loc[:]], outs=[hal_full[:]],
                          replica_groups=rg)

                  # ---------- edge phase
                  for w in range(n_win):
                      tA, tB = cfg.ta[w], cfg.tb[w]
                      t = tA + tB
                      idx_t = sm.tile([P, t_max * 8], mybir.dt.int16,
                                      tag="idx_t")
                      nc.scalar.dma_start(out=idx_t[:, : t * 8],
                                          in_=idx16[w, :, : t * 8])
                      dl_t = sm.tile([P, t_max], mybir.dt.int32, tag="dl_t")
                      nc.scalar.dma_start(out=dl_t[:, :t], in_=dst32[w, :, :t])

                      gath = gp.tile([P, t_max, lc.drow], bf16,
                                     tag=f"gath{lc.drow}")
                      nc.gpsimd.dma_gather(
                          gath[:, :tA, :], hal_full[: cfg.half, :],
                          idx_t[:, : tA * 8],
                          num_idxs=tA * P, num_idxs_reg=tA * P,
                          elem_size=lc.drow, single_packet=False)
                      nc.gpsimd.dma_gather(
                          gath[:, tA:t, :], hal_full[cfg.half :, :],
                          idx_t[:, tA * 8 : t * 8],
                          num_idxs=tB * P, num_idxs_reg=tB * P,
                          elem_size=lc.drow, single_packet=False)
                      # one-hot M for the whole window
                      dl_bf = sm.tile([P, t_max], bf16, tag="dl_bf")
                      nc.vector.tensor_copy(out=dl_bf[:, :t], in_=dl_t[:, :t])
                      m_all = sb.tile([P, t_max * P], bf16, tag="m_all")
                      nc.vector.tensor_tensor(
                          out=m_all[:, : t * P].rearrange(
                              "p (t n) -> p t n", t=t),
                          in0=iota_bf[:].unsqueeze(1).to_broadcast([P, t, P]),
                          in1=dl_bf[:, :t].unsqueeze(2).to_broadcast([P, t, P]),
                          op=mybir.AluOpType.is_equal)

                      # ald[dst] per edge via PE: Mt_j @ ald_win
                      aldps = psf.tile([P, t_max * 8], f32, tag="aldps")
                      for j in range(t):
                          mt_ps = psf.tile([P, P], bf16, tag="tp")
                          nc.tensor.transpose(
                              mt_ps[:], m_all[:, j * P : (j + 1) * P],
                              identity[:])
                          mt_sb = sb.tile([P, P], bf16, tag="mt_sb")
                          nc.scalar.copy(out=mt_sb[:], in_=mt_ps[:])
                          nc.tensor.matmul(
                              aldps[:, j * lc.H : (j + 1) * lc.H],
                              lhsT=mt_sb[:],
                              rhs=ald_all[:, w * lc.H : (w + 1) * lc.H],
                              start=True, stop=True)

                      # e0 = als[src] + ald[dst]; exp(leaky) via max(x, 0.2x)
                      e0 = sm.tile([P, t_max * 8], f32, tag="e0")
                      nc.vector.tensor_tensor(
                          out=e0[:, : t * lc.H].rearrange(
                              "p (t h) -> p t h", t=t),
                          in0=gath[:, :t, lc.fout : lc.fout + lc.H],
                          in1=aldps[:, : t * lc.H].rearrange(
                              "p (t h) -> p t h", t=t),
                          op=mybir.AluOpType.add)
                      t1 = sm.tile([P, t_max * 8], f32, tag="t1")
                      nc.vector.tensor_scalar(
                          out=t1[:, : t * lc.H], in0=e0[:, : t * lc.H],
                          scalar1=NEG_SLOPE, scalar2=None,
                          op0=mybir.AluOpType.mult)
                      nc.vector.tensor_tensor(
                          out=e0[:, : t * lc.H], in0=e0[:, : t * lc.H],
                          in1=t1[:, : t * lc.H], op=mybir.AluOpType.max)
                      nc.scalar.activation(
                          gath[:, :t, lc.fout : lc.fout + lc.H],
                          e0[:, : t * lc.H].rearrange("p (t h) -> p t h", t=t),
                          mybir.ActivationFunctionType.Exp)

                      # msg scale: h *= exp_e (c-major h: broadcast over the
                      # middle C dim keeps the last dim packed for DVE 2x)
                      h_view = gath[:, :t, : lc.fout].rearrange(
                          "p t (c h) -> p t c h", c=lc.C)
                      expv = gath[:, :t, lc.fout : lc.fout + lc.H].unsqueeze(
                          2).to_broadcast([P, t, lc.C, lc.H])
                      nc.vector.tensor_tensor(out=h_view, in0=h_view, in1=expv,
                                              op=mybir.AluOpType.mult)

                      # scatter-add via PE
                      acc = ps.tile([P, 264], f32, tag="acc")
                      for j in range(t):
                          nc.tensor.matmul(
                              acc[:, : lc.fout + lc.H],
                              lhsT=m_all[:, j * P : (j + 1) * P],
                              rhs=gath[:, j, : lc.fout + lc.H],
                              start=(j == 0), stop=(j == t - 1))

                      # normalize + bias (+relu) at node level
                      sden = sm.tile([P, 8], f32, tag="sden")
                      nc.vector.tensor_scalar(
                          out=sden[:, : lc.H],
                          in0=acc[:, lc.fout : lc.fout + lc.H],
                          scalar1=1e-12, scalar2=None, op0=mybir.AluOpType.max)
                      rec = sm.tile([P, 8], f32, tag="rec")
                      nc.vector.reciprocal(rec[:, : lc.H], sden[:, : lc.H])
                      recv = rec[:, : lc.H].unsqueeze(1).to_broadcast(
                          [P, lc.C, lc.H])
                      o_t = fr.tile([P, 256], f32, tag="o_t")
                      nc.vector.tensor_tensor(
                          out=o_t[:, : lc.fout].rearrange(
                              "p (c h) -> p c h", c=lc.C),
                          in0=acc[:, : lc.fout].rearrange(
                              "p (c h) -> p c h", c=lc.C),
                          in1=recv, op=mybir.AluOpType.mult)
                      nc.vector.tensor_tensor(
                          out=o_t[:, : lc.fout], in0=o_t[:, : lc.fout],
                          in1=b_bc[:], op=mybir.AluOpType.add)
                      if lc.relu:
                          nc.vector.tensor_scalar(
                              out=in_local[:, w, : lc.fout],
                              in0=o_t[:, : lc.fout],
                              scalar1=0.0, scalar2=None,
                              op0=mybir.AluOpType.max)
                      else:
                          nc.sync.dma_start(
                              out=out_t[w * P : (w + 1) * P, :],
                              in_=o_t[:, : lc.fout])

    nc.compile()
    return nc


# ---------------------------------------------------------------- runner

def _make_pjrt_fn(nc, n_cores):
    """Cached PJRT executable for nc (modeled on bass2jax.run_bass_via_pjrt,
    without output-buffer donation so it can be re-invoked for timing)."""
    import jax
    from jax.sharding import Mesh, PartitionSpec
    from jax.experimental.shard_map import shard_map
    from concourse import bass2jax, mybir as mb

    bass2jax.install_neuronx_cc_hook()
    partition_name = nc.partition_id_tensor.name if nc.partition_id_tensor else None
    in_names, out_names, out_avals, zero_outs = [], [], [], []
    for alloc in nc.m.functions[0].allocations:
        if not isinstance(alloc, mb.MemoryLocationSet):
            continue
        name = alloc.memorylocations[0].name
        if alloc.kind == "ExternalInput":
            if name != partition_name:
                in_names.append(name)
        elif alloc.kind == "ExternalOutput":
            out_names.append(name)
            shape = tuple(alloc.tensor_shape)
            dtype = mb.dt.np(alloc.dtype)
            out_avals.append(jax.core.ShapedArray(shape, dtype))
            zero_outs.append(np.zeros(shape, dtype))
    n_params = len(in_names)
    all_in_names = list(in_names) + list(out_names)
    if partition_name is not None:
        all_in_names.append(partition_name)

    def _body(*args):
        operands = list(args)
        if partition_name is not None:
            operands.append(bass2jax.partition_id_tensor())
        outs = bass2jax._bass_exec_p.bind(
            *operands,
            out_avals=tuple(out_avals),
            in_names=tuple(all_in_names),
            out_names=tuple(out_names),
            lowering_input_output_aliases=(),
            sim_require_finite=True,
            sim_require_nnan=True,
            nc=nc,
        )
        return tuple(outs)

    devices = jax.devices()[:n_cores]
    mesh = Mesh(np.asarray(devices), ("core",))
    n_outs = len(out_avals)
    in_specs = (PartitionSpec("core"),) * (n_params + n_outs)
    out_specs = (PartitionSpec("core"),) * n_outs
    fn = jax.jit(shard_map(_body, mesh=mesh, in_specs=in_specs,
                           out_specs=out_specs, check_rep=False),
                 keep_unused=True)
    return fn, in_names, out_names, out_avals, zero_outs


def run(cfg: GatCfg, inputs, time_iters=0, repeats=1, in_maps=None):
    """Returns (out, best_exec_seconds or None)."""
    import time as _time
    import jax

    if in_maps is None:
        in_maps = shard_inputs(cfg, inputs)  # sets cfg.ta / tb
    nc = build_program(cfg, repeats=repeats)
    n_cores = cfg.n_cores
    fn, in_names, out_names, out_avals, zero_outs = _make_pjrt_fn(nc, n_cores)

    concat_in = [
        np.concatenate([np.asarray(in_maps[c][name]) for c in range(n_cores)], axis=0)
        for name in in_names
    ]
    concat_zero = [
        np.zeros((n_cores * z.shape[0], *z.shape[1:]), z.dtype) for z in zero_outs
    ]
    dev_in = [jax.device_put(a) for a in concat_in]
    dev_zero = [jax.device_put(a) for a in concat_zero]

    out_arrs = fn(*dev_in, *dev_zero)
    jax.block_until_ready(out_arrs)

    best = None
    if time_iters:
        times = []
        for _ in range(time_iters):
            t0 = _time.perf_counter()
            out_arrs2 = fn(*dev_in, *dev_zero)
            jax.block_until_ready(out_arrs2)
            times.append(_time.perf_counter() - t0)
        best = min(times)

    oi = out_names.index("out")
    full = np.asarray(out_arrs[oi]).reshape(n_cores, *out_avals[oi].shape)
    got = np.concatenate(list(full), axis=0)
    out = unshard_output(cfg, got)
    return out, best


# ---------------------------------------------------------------- entry point

def kernel(**inputs):
    """Full-input GAT kernel: shards across 8 NeuronCores internally,
    runs the Bass program via run_bass_kernel_spmd, returns [50000, 64] f32."""
    from concourse.bass_utils import run_bass_kernel_spmd

    cfg = real_cfg()
    in_maps = shard_inputs(cfg, inputs)  # sets cfg.ta/tb from edge_index
    nc = build_program(cfg)
    res = run_bass_kernel_spmd(nc, in_maps, core_ids=list(range(cfg.n_cores)))
    got = np.concatenate(
        [res.results[c]["out"] for c in range(cfg.n_cores)], axis=0)
    return unshard_output(cfg, got).astype(np.float32)


# revision 11
# speedup vs baseline: 1.8839x; 1.8839x over previous
"""3-layer GAT on Trainium2 — 8-core SPMD Bass kernel (bf16 pipeline).

Node-partitioned (edge-cut) distribution:
- core c owns nodes [c*NPC, (c+1)*NPC); nodes are re-ordered within each core
  (degree-sorted snake) so the 49 windows of 128 dst nodes carry near-equal
  edge counts.
- per layer: each core computes its nodes' [h | als] rows (bf16) plus a local
  ald tile, publishes rows to a replicated table via AllGather, then processes
  its incoming edges in windows: dma_gather of source rows (bf16),
  exp(leaky(als+ald)) on-chip, scatter-add via one-hot matmul into PSUM,
  normalize by the per-node sum at the end.
- all tables, gathers and matmuls run in bf16 (f32 PSUM accumulation).
"""
from dataclasses import dataclass, field

import numpy as np
import ml_dtypes

import concourse.bass as bass
import concourse.bacc as bacc
import concourse.mybir as mybir
import concourse.tile as tile
from concourse.masks import make_identity

P = 128
N_CORES = 8
NEG_SLOPE = 0.2
DUMMY_ALS = -1000.0

BF16 = ml_dtypes.bfloat16


@dataclass
class LayerCfg:
    fin: int
    fout: int
    H: int
    C: int
    drow: int      # table row width in bf16 elements (multiple of 128)
    relu: bool

    @property
    def ex(self):
        return 2 * self.H

    @property
    def fw(self):
        return self.fout + self.ex


@dataclass
class GatCfg:
    n: int
    n_cores: int = N_CORES
    layers: tuple = ()
    # filled by preprocessing
    ta: tuple = ()          # per-window A-half slot columns (max over cores)
    tb: tuple = ()          # per-window B-half slot columns
    newpos: np.ndarray = None   # global node -> position within its core
    ready: bool = False

    @property
    def npc(self):
        return self.n // self.n_cores

    @property
    def n_win(self):
        return (self.npc + P - 1) // P

    @property
    def pos(self):
        return self.n_win * P

    @property
    def rows(self):
        return self.pos + 1

    @property
    def half(self):
        return self.rows * (self.n_cores // 2)

    @property
    def tbl(self):
        return self.rows * self.n_cores

    @property
    def t_max(self):
        return max(a + b for a, b in zip(self.ta, self.tb))


def real_cfg():
    return GatCfg(
        n=50000,
        layers=(
            LayerCfg(256, 256, 8, 32, 384, True),
            LayerCfg(256, 256, 8, 32, 384, True),
            LayerCfg(256, 64, 1, 64, 128, False),
        ),
    )


def mini_cfg():
    return GatCfg(
        n=1024,
        layers=(
            LayerCfg(256, 256, 8, 32, 384, True),
            LayerCfg(256, 256, 8, 32, 384, True),
            LayerCfg(256, 64, 1, 64, 128, False),
        ),
    )


# ---------------------------------------------------------------- host prep

def _wrap16(idx_list):
    """dma_gather index layout: idx i lives at [i % 16, i // 16]; replicate
    the 16-partition block 8x down to 128 partitions."""
    a = idx_list.reshape(-1, 16).T  # [16, len/16]
    return np.tile(a, (8, 1)).astype(np.int16)


def preprocess(cfg: GatCfg, edge_index):
    """Degree-balanced node->window assignment (snake over degree-sorted
    nodes), then per-window column-major edge packing split by src half.

    Sets cfg.ta/tb/newpos; returns per-core index inputs.
    """
    npc, n_win, n_cores = cfg.npc, cfg.n_win, cfg.n_cores
    src = np.concatenate([np.asarray(edge_index[0], np.int64),
                          np.arange(cfg.n, dtype=np.int64)])
    dst = np.concatenate([np.asarray(edge_index[1], np.int64),
                          np.arange(cfg.n, dtype=np.int64)])
    dst_core = dst // npc

    # --- node placement: snake over degree-sorted nodes, per core
    newpos = np.empty(cfg.n, dtype=np.int64)
    for c in range(n_cores):
        sel = dst_core == c
        d_loc = dst[sel] - c * npc
        deg = np.bincount(d_loc, minlength=npc)
        order = np.argsort(-deg, kind="stable")  # old local, degree desc
        r = np.arange(npc)
        k, j = r // n_win, r % n_win
        w = np.where(k % 2 == 0, j, n_win - 1 - j)
        newpos[c * npc + order] = w * P + k
    cfg.newpos = newpos

    rows, half = cfg.rows, cfg.half
    sidx = (src // npc) * rows + newpos[src]
    is_b = sidx >= half

    # --- per-core edge grouping
    per_core = []
    counts_a = np.zeros((n_cores, n_win), dtype=np.int64)
    counts_b = np.zeros((n_cores, n_win), dtype=np.int64)
    for c in range(n_cores):
        sel = np.nonzero(dst_core == c)[0]
        d_new = newpos[dst[sel]]
        w = d_new // P
        half_flag = is_b[sel].astype(np.int64)
        order = np.lexsort((d_new, half_flag, w))
        sel = sel[order]
        d_new = d_new[order]
        w = w[order]
        half_flag = half_flag[order]
        rel = (sidx[sel] - half_flag * half).astype(np.int64)
        for wi in range(n_win):
            m = w == wi
            counts_a[c, wi] = int((m & (half_flag == 0)).sum())
            counts_b[c, wi] = int((m & (half_flag == 1)).sum())
        per_core.append((d_new % P, w, half_flag, rel))

    ta = np.maximum(1, np.ceil(counts_a.max(axis=0) / P).astype(int))
    tb = np.maximum(1, np.ceil(counts_b.max(axis=0) / P).astype(int))
    cfg.ta, cfg.tb = tuple(int(x) for x in ta), tuple(int(x) for x in tb)
    cfg.ready = True
    t_max = cfg.t_max
    dummy = cfg.pos  # dummy row index, valid in A-rel and B-rel coords

    idx_inputs = []
    for c in range(n_cores):
        d_loc, w, half_flag, rel = per_core[c]
        idx16 = np.zeros((n_win, P, t_max * 8), dtype=np.int16)
        dst32 = np.zeros((n_win, P, t_max), dtype=np.int32)
        for wi in range(n_win):
            tA, tB = cfg.ta[wi], cfg.tb[wi]
            t = tA + tB
            m = w == wi
            ra = rel[m & (half_flag == 0)]
            rb = rel[m & (half_flag == 1)]
            da = d_loc[m & (half_flag == 0)]
            db = d_loc[m & (half_flag == 1)]
            src_list = np.full(t * P, dummy, dtype=np.int64)
            dl_list = np.zeros(t * P, dtype=np.int64)
            src_list[: len(ra)] = ra
            src_list[tA * P : tA * P + len(rb)] = rb
            dl_list[: len(da)] = da
            dl_list[tA * P : tA * P + len(db)] = db
            wa = _wrap16(src_list[: tA * P])
            wb = _wrap16(src_list[tA * P :])
            idx16[wi, :, : t * 8] = np.concatenate([wa, wb], axis=1)
            # dstloc in (p, j) layout: edge i -> [i % 128, i // 128]
            dst32[wi, :, :t] = dl_list.reshape(t, P).T.astype(np.int32)
        idx_inputs.append({"idx16": idx16, "dst32": dst32})
    return idx_inputs


def shard_inputs(cfg: GatCfg, inputs):
    """Build the per-core input dicts for run_bass_kernel_spmd."""
    x = np.asarray(inputs["x"], dtype=np.float32)
    edge_index = np.asarray(inputs["edge_index"])
    idx_inputs = preprocess(cfg, edge_index)

    def blockdiag(a_s, a_d, fin_rows):
        H, C = a_s.shape
        A = np.zeros((fin_rows, 2 * H), dtype=np.float64)
        for h in range(H):
            A[h * C : (h + 1) * C, h] = a_s[h]
            A[h * C : (h + 1) * C, H + h] = a_d[h]
        return A

    # Device activations are stored head-minor ("c-major": feature (h,c) lives
    # at column c*H + h) so the per-edge exp broadcast multiplies with a
    # packed last dim (DVE 2x mode). Weight columns/rows are permuted here.
    weight_common = {}
    pm_prev = None
    for li, nm in enumerate(["1", "2", "3"]):
        lc = cfg.layers[li]
        pm = np.array([h * lc.C + c for c in range(lc.C) for h in range(lc.H)])
        Wn = np.asarray(inputs[f"W{nm}"], dtype=np.float64)
        if pm_prev is not None:
            Wn = Wn[pm_prev, :]
        A = blockdiag(
            np.asarray(inputs[f"as{nm}"], np.float64),
            np.asarray(inputs[f"ad{nm}"], np.float64),
            lc.fout,
        )
        WA = Wn @ A  # [fin, 2H]
        Wx = np.concatenate([Wn[:, pm], WA], axis=1)  # [fin, fout + 2H]
        weight_common[f"Wx{nm}"] = np.ascontiguousarray(Wx.astype(BF16))
        weight_common[f"b{nm}"] = np.asarray(
            inputs[f"b{nm}"], np.float32).reshape(1, -1)[:, pm]
        pm_prev = pm

    in_maps = []
    for c in range(cfg.n_cores):
        xs = np.zeros((cfg.pos, cfg.layers[0].fin), dtype=BF16)
        loc = cfg.newpos[c * cfg.npc : (c + 1) * cfg.npc]
        xs[loc] = x[c * cfg.npc : (c + 1) * cfg.npc].astype(BF16)
        m = {"x_sh": xs, **idx_inputs[c], **weight_common}
        in_maps.append(m)
    return in_maps


def unshard_output(cfg: GatCfg, got):
    """Map the concatenated per-core outputs back to global node order."""
    out = np.empty((cfg.n, got.shape[1]), dtype=got.dtype)
    for c in range(cfg.n_cores):
        loc = cfg.newpos[c * cfg.npc : (c + 1) * cfg.npc]
        out[c * cfg.npc : (c + 1) * cfg.npc] = got[c * cfg.pos + loc]
    return out


# ---------------------------------------------------------------- device code

def build_program(cfg: GatCfg, repeats: int = 1, skip_ag: bool = False):
    assert cfg.ready, "run preprocess/shard_inputs first (sets ta/tb)"
    nc = bacc.Bacc("TRN2", target_bir_lowering=False, debug=False,
                   num_devices=cfg.n_cores)
    n_win, t_max = cfg.n_win, cfg.t_max
    f32 = mybir.dt.float32
    bf16 = mybir.dt.bfloat16

    # ---- I/O
    x_sh = nc.dram_tensor("x_sh", [cfg.pos, cfg.layers[0].fin], bf16,
                          kind="ExternalInput")
    idx16 = nc.dram_tensor("idx16", [n_win, P, t_max * 8], mybir.dt.int16,
                           kind="ExternalInput")
    dst32 = nc.dram_tensor("dst32", [n_win, P, t_max], mybir.dt.int32,
                           kind="ExternalInput")
    wt_in = {}
    for li, nm in enumerate(["1", "2", "3"]):
        lc = cfg.layers[li]
        wt_in[f"Wx{nm}"] = nc.dram_tensor(f"Wx{nm}", [lc.fin, lc.fw], bf16,
                                          kind="ExternalInput")
        wt_in[f"b{nm}"] = nc.dram_tensor(f"b{nm}", [1, lc.fout], f32,
                                         kind="ExternalInput")
    out_t = nc.dram_tensor("out", [cfg.pos, cfg.layers[-1].fout], f32,
                           kind="ExternalOutput")

    # ---- internal DRAM
    hal12_loc = nc.dram_tensor("hal12_loc", [cfg.rows, 384], bf16,
                               kind="Internal")
    hal12_full = nc.dram_tensor("hal12_full", [cfg.tbl, 384], bf16,
                                kind="Internal", addr_space="Shared")
    hal3_loc = nc.dram_tensor("hal3_loc", [cfg.rows, 128], bf16,
                              kind="Internal")
    hal3_full = nc.dram_tensor("hal3_full", [cfg.tbl, 128], bf16,
                               kind="Internal", addr_space="Shared")

    iota_const = nc.inline_tensor(
        np.tile(np.arange(P, dtype=np.float32), (P, 1)), name="iota_const")

    rg = [list(range(cfg.n_cores))]

    with tile.TileContext(nc) as tc:
        import contextlib
        with contextlib.ExitStack() as ctx:
            persist = ctx.enter_context(tc.tile_pool(name="persist", bufs=1))
            wpool = ctx.enter_context(tc.tile_pool(name="wts", bufs=1))
            sb = ctx.enter_context(tc.tile_pool(name="work", bufs=2))
            gp = ctx.enter_context(tc.tile_pool(name="gathp", bufs=3))
            sm = ctx.enter_context(tc.tile_pool(name="small", bufs=4))
            fr = ctx.enter_context(tc.tile_pool(name="front", bufs=2))
            ps = ctx.enter_context(tc.tile_pool(name="psum", bufs=2, space="PSUM"))
            psf = ctx.enter_context(tc.tile_pool(name="psumf", bufs=2, space="PSUM"))

            identity = persist.tile([P, P], bf16)
            make_identity(nc, identity[:])
            iota_f32 = persist.tile([P, P], f32)
            nc.sync.dma_start(out=iota_f32[:], in_=iota_const[:])
            iota_bf = persist.tile([P, P], bf16)
            nc.vector.tensor_copy(out=iota_bf[:], in_=iota_f32[:])

            in_local = persist.tile([P, n_win, cfg.layers[0].fin], bf16)
            if skip_ag:
                # one-time zero fill of the tables so gathers read finite
                # data (timing-only variant; results are wrong by design)
                z = persist.tile([P, 384], bf16)
                nc.vector.memset(z[:], 0.0)
                for _hal, _dr in ((hal12_full, 384), (hal3_full, 128)):
                    nb = cfg.tbl // P * P
                    nc.sync.dma_start(
                        out=_hal[:nb, :].rearrange("(r p) d -> p r d", p=P),
                        in_=z[:, :_dr].unsqueeze(1).to_broadcast(
                            [P, nb // P, _dr]))
                    nc.sync.dma_start(out=_hal[nb:, :],
                                      in_=z[: cfg.tbl - nb, :_dr])
            for _rep in range(repeats):
              for w in range(n_win):
                nc.sync.dma_start(out=in_local[:, w, :],
                                  in_=x_sh[w * P : (w + 1) * P, :])

              for li in range(len(cfg.layers)):
                  lc = cfg.layers[li]
                  nm = str(li + 1)
                  hal_loc = hal12_loc if lc.drow == 384 else hal3_loc
                  hal_full = hal12_full if lc.drow == 384 else hal3_full
                  kch = lc.fin // P  # input chunks (contraction)

                  # ---------- per-layer constants
                  w_ext = wpool.tile([P, kch, 272], bf16, tag="w_ext")
                  nc.sync.dma_start(
                      out=w_ext[:, :, : lc.fw],
                      in_=wt_in[f"Wx{nm}"][:].rearrange("(q p) f -> p q f", p=P))
                  ald_all = wpool.tile([P, n_win * 8], bf16, tag="ald_all")
                  b_t = sm.tile([1, lc.fout], f32, tag="b_t")
                  nc.sync.dma_start(out=b_t[:], in_=wt_in[f"b{nm}"][:])
                  b_bc = wpool.tile([P, lc.fout], f32, tag="b_bc")
                  nc.gpsimd.partition_broadcast(b_bc[:], b_t[:1, :])

                  # ---------- front phase: [h | als] for own nodes -> hal_loc
                  for w in range(n_win):
                      in_t = fr.tile([P, lc.fin], bf16, tag="in_t")
                      for q in range(kch):
                          tp = psf.tile([P, P], bf16, tag="tp")
                          nc.tensor.transpose(
                              tp[:], in_local[:, w, q * P : (q + 1) * P],
                              identity[:])
                          nc.vector.tensor_copy(
                              out=in_t[:, q * P : (q + 1) * P], in_=tp[:])
                      h_ps = psf.tile([P, 272], f32, tag="h_ps")
                      for q in range(kch):
                          nc.tensor.matmul(
                              h_ps[:, : lc.fw],
                              lhsT=in_t[:, q * P : (q + 1) * P],
                              rhs=w_ext[:, q, : lc.fw],
                              start=(q == 0), stop=(q == kch - 1))
                      nc.scalar.copy(
                          out=ald_all[:, w * lc.H : (w + 1) * lc.H],
                          in_=h_ps[:, lc.fout + lc.H : lc.fout + 2 * lc.H])
                      stage = fr.tile([P, lc.drow], bf16, tag=f"stage{lc.drow}")
                      nc.vector.tensor_copy(
                          out=stage[:, : lc.fout + lc.H],
                          in_=h_ps[:, : lc.fout + lc.H])
                      nc.sync.dma_start(
                          out=hal_loc[w * P : (w + 1) * P, : lc.fout + lc.H],
                          in_=stage[:, : lc.fout + lc.H])
                  # dummy row
                  dmy = sm.tile([1, lc.drow], bf16, tag=f"dmy{lc.drow}")
                  nc.vector.memset(dmy[:], 0.0)
                  nc.vector.memset(
                      dmy[:1, lc.fout : lc.fout + lc.H], DUMMY_ALS)
                  nc.sync.dma_start(out=hal_loc[cfg.pos : cfg.pos + 1, :],
                                    in_=dmy[:])

                  if not skip_ag:
                      nc.gpsimd.collective_compute(
                          "AllGather", mybir.AluOpType.bypass,
                          ins=[hal_loc[:]], outs=[hal_full[:]],
                          replica_groups=rg)

                  # ---------- edge phase
                  for w in range(n_win):
                      tA, tB = cfg.ta[w], cfg.tb[w]
                      t = tA + tB
                      idx_t = sm.tile([P, t_max * 8], mybir.dt.int16,
                                      tag="idx_t")
                      nc.scalar.dma_start(out=idx_t[:, : t * 8],
                                          in_=idx16[w, :, : t * 8])
                      dl_t = sm.tile([P, t_max], mybir.dt.int32, tag="dl_t")
                      nc.scalar.dma_start(out=dl_t[:, :t], in_=dst32[w, :, :t])

                      gath = gp.tile([P, t_max, lc.drow], bf16,
                                     tag=f"gath{lc.drow}")
                      nc.gpsimd.dma_gather(
                          gath[:, :tA, :], hal_full[: cfg.half, :],
                          idx_t[:, : tA * 8],
                          num_idxs=tA * P, num_idxs_reg=tA * P,
                          elem_size=lc.drow, single_packet=False)
                      nc.gpsimd.dma_gather(
                          gath[:, tA:t, :], hal_full[cfg.half :, :],
                          idx_t[:, tA * 8 : t * 8],
                          num_idxs=tB * P, num_idxs_reg=tB * P,
                          elem_size=lc.drow, single_packet=False)
                      # one-hot M for the whole window
                      dl_bf = sm.tile([P, t_max], bf16, tag="dl_bf")
                      nc.vector.tensor_copy(out=dl_bf[:, :t], in_=dl_t[:, :t])
                      m_all = sb.tile([P, t_max * P], bf16, tag="m_all")
                      nc.vector.tensor_tensor(
                          out=m_all[:, : t * P].rearrange(
                              "p (t n) -> p t n", t=t),
                          in0=iota_bf[:].unsqueeze(1).to_broadcast([P, t, P]),
                          in1=dl_bf[:, :t].unsqueeze(2).to_broadcast([P, t, P]),
                          op=mybir.AluOpType.is_equal)

                      # ald[dst] per edge via PE: Mt_j @ ald_win
                      aldps = psf.tile([P, t_max * 8], f32, tag="aldps")
                      for j in range(t):
                          mt_ps = psf.tile([P, P], bf16, tag="tp")
                          nc.tensor.transpose(
                              mt_ps[:], m_all[:, j * P : (j + 1) * P],
                              identity[:])
                          mt_sb = sb.tile([P, P], bf16, tag="mt_sb")
                          nc.scalar.copy(out=mt_sb[:], in_=mt_ps[:])
                          nc.tensor.matmul(
                              aldps[:, j * lc.H : (j + 1) * lc.H],
                              lhsT=mt_sb[:],
                              rhs=ald_all[:, w * lc.H : (w + 1) * lc.H],
                              start=True, stop=True)

                      # e0 = als[src] + ald[dst]; exp(leaky) via max(x, 0.2x)
                      e0 = sm.tile([P, t_max * 8], f32, tag="e0")
                      nc.vector.tensor_tensor(
                          out=e0[:, : t * lc.H].rearrange(
                              "p (t h) -> p t h", t=t),
                          in0=gath[:, :t, lc.fout : lc.fout + lc.H],
                          in1=aldps[:, : t * lc.H].rearrange(
                              "p (t h) -> p t h", t=t),
                          op=mybir.AluOpType.add)
                      t1 = sm.tile([P, t_max * 8], f32, tag="t1")
                      nc.vector.tensor_scalar(
                          out=t1[:, : t * lc.H], in0=e0[:, : t * lc.H],
                          scalar1=NEG_SLOPE, scalar2=None,
                          op0=mybir.AluOpType.mult)
                      nc.vector.tensor_tensor(
                          out=e0[:, : t * lc.H], in0=e0[:, : t * lc.H],
                          in1=t1[:, : t * lc.H], op=mybir.AluOpType.max)
                      nc.scalar.activation(
                          gath[:, :t, lc.fout : lc.fout + lc.H],
                          e0[:, : t * lc.H].rearrange("p (t h) -> p t h", t=t),
                          mybir.ActivationFunctionType.Exp)

                      # msg scale: h *= exp_e (c-major h: broadcast over the
                      # middle C dim keeps the last dim packed for DVE 2x)
                      h_view = gath[:, :t, : lc.fout].rearrange(
                          "p t (c h) -> p t c h", c=lc.C)
                      expv = gath[:, :t, lc.fout : lc.fout + lc.H].unsqueeze(
                          2).to_broadcast([P, t, lc.C, lc.H])
                      nc.vector.tensor_tensor(out=h_view, in0=h_view, in1=expv,
                                              op=mybir.AluOpType.mult)

                      # scatter-add via PE
                      acc = ps.tile([P, 264], f32, tag="acc")
                      for j in range(t):
                          nc.tensor.matmul(
                              acc[:, : lc.fout + lc.H],
                              lhsT=m_all[:, j * P : (j + 1) * P],
                              rhs=gath[:, j, : lc.fout + lc.H],
                              start=(j == 0), stop=(j == t - 1))

                      # normalize + bias (+relu) at node level
                      sden = sm.tile([P, 8], f32, tag="sden")
                      nc.vector.tensor_scalar(
                          out=sden[:, : lc.H],
                          in0=acc[:, lc.fout : lc.fout + lc.H],
                          scalar1=1e-12, scalar2=None, op0=mybir.AluOpType.max)
                      rec = sm.tile([P, 8], f32, tag="rec")
                      nc.vector.reciprocal(rec[:, : lc.H], sden[:, : lc.H])
                      recv = rec[:, : lc.H].unsqueeze(1).to_broadcast(
                          [P, lc.C, lc.H])
                      o_t = fr.tile([P, 256], f32, tag="o_t")
                      nc.vector.tensor_tensor(
                          out=o_t[:, : lc.fout].rearrange(
                              "p (c h) -> p c h", c=lc.C),
                          in0=acc[:, : lc.fout].rearrange(
                              "p (c h) -> p c h", c=lc.C),
                          in1=recv, op=mybir.AluOpType.mult)
                      nc.vector.tensor_tensor(
                          out=o_t[:, : lc.fout], in0=o_t[:, : lc.fout],
                          in1=b_bc[:], op=mybir.AluOpType.add)
                      if lc.relu:
                          nc.vector.tensor_scalar(
                              out=in_local[:, w, : lc.fout],
                              in0=o_t[:, : lc.fout],
                              scalar1=0.0, scalar2=None,
                              op0=mybir.AluOpType.max)
                      else:
                          nc.sync.dma_start(
                              out=out_t[w * P : (w + 1) * P, :],
                              in_=o_t[:, : lc.fout])

    nc.compile()
    return nc


# ---------------------------------------------------------------- runner

def _make_pjrt_fn(nc, n_cores):
    """Cached PJRT executable for nc (modeled on bass2jax.run_bass_via_pjrt,
    without output-buffer donation so it can be re-invoked for timing)."""
    import jax
    from jax.sharding import Mesh, PartitionSpec
    from jax.experimental.shard_map import shard_map
    from concourse import bass2jax, mybir as mb

    bass2jax.install_neuronx_cc_hook()
    partition_name = nc.partition_id_tensor.name if nc.partition_id_tensor else None
    in_names, out_names, out_avals, zero_outs = [], [], [], []
    for alloc in nc.m.functions[0].allocations:
        if not isinstance(alloc, mb.MemoryLocationSet):
            continue
        name = alloc.memorylocations[0].name
        if alloc.kind == "ExternalInput":
            if name != partition_name:
                in_names.append(name)
        elif alloc.kind == "ExternalOutput":
            out_names.append(name)
            shape = tuple(alloc.tensor_shape)
            dtype = mb.dt.np(alloc.dtype)
            out_avals.append(jax.core.ShapedArray(shape, dtype))
            zero_outs.append(np.zeros(shape, dtype))
    n_params = len(in_names)
    all_in_names = list(in_names) + list(out_names)
    if partition_name is not None:
        all_in_names.append(partition_name)

    def _body(*args):
        operands = list(args)
        if partition_name is not None:
            operands.append(bass2jax.partition_id_tensor())
        outs = bass2jax._bass_exec_p.bind(
            *operands,
            out_avals=tuple(out_avals),
            in_names=tuple(all_in_names),
            out_names=tuple(out_names),
            lowering_input_output_aliases=(),
            sim_require_finite=True,
            sim_require_nnan=True,
            nc=nc,
        )
        return tuple(outs)

    devices = jax.devices()[:n_cores]
    mesh = Mesh(np.asarray(devices), ("core",))
    n_outs = len(out_avals)
    in_specs = (PartitionSpec("core"),) * (n_params + n_outs)
    out_specs = (PartitionSpec("core"),) * n_outs
    fn = jax.jit(shard_map(_body, mesh=mesh, in_specs=in_specs,
                           out_specs=out_specs, check_rep=False),
                 keep_unused=True)
    return fn, in_names, out_names, out_avals, zero_outs


def run(cfg: GatCfg, inputs, time_iters=0, repeats=1, in_maps=None):
    """Returns (out, best_exec_seconds or None)."""
    import time as _time
    import jax

    if in_maps is None:
        in_maps = shard_inputs(cfg, inputs)  # sets cfg.ta / tb
    nc = build_program(cfg, repeats=repeats)
    n_cores = cfg.n_cores
    fn, in_names, out_names, out_avals, zero_outs = _make_pjrt_fn(nc, n_cores)

    concat_in = [
        np.concatenate([np.asarray(in_maps[c][name]) for c in range(n_cores)], axis=0)
        for name in in_names
    ]
    concat_zero = [
        np.zeros((n_cores * z.shape[0], *z.shape[1:]), z.dtype) for z in zero_outs
    ]
    dev_in = [jax.device_put(a) for a in concat_in]
    dev_zero = [jax.device_put(a) for a in concat_zero]

    out_arrs = fn(*dev_in, *dev_zero)
    jax.block_until_ready(out_arrs)

    best = None
    if time_iters:
        times = []
        for _ in range(time_iters):
            t0 = _time.perf_counter()
            out_arrs2 = fn(*dev_in, *dev_zero)
            jax.block_until_ready(out_arrs2)
            times.append(_time.perf_counter() - t0)
        best = min(times)

    oi = out_names.index("out")
    full = np.asarray(out_arrs[oi]).reshape(n_cores, *out_avals[oi].shape)
    got = np.concatenate(list(full), axis=0)
    out = unshard_output(cfg, got)
    return out, best


# ---------------------------------------------------------------- entry point

def kernel(**inputs):
    """Full-input GAT kernel: shards across 8 NeuronCores internally,
    runs the Bass program via run_bass_kernel_spmd, returns [50000, 64] f32."""
    from concourse.bass_utils import run_bass_kernel_spmd

    cfg = real_cfg()
    in_maps = shard_inputs(cfg, inputs)  # sets cfg.ta/tb from edge_index
    nc = build_program(cfg)
    res = run_bass_kernel_spmd(nc, in_maps, core_ids=list(range(cfg.n_cores)))
    got = np.concatenate(
        [res.results[c]["out"] for c in range(cfg.n_cores)], axis=0)
    return unshard_output(cfg, got).astype(np.float32)
